# revision 16
# baseline (speedup 1.0000x reference)
"""ActionRelationEncoder Bass kernel for 8 Trainium2 NeuronCores.

Data-parallel over batch (B=64), weights replicated on every core. The
wall-clock bottleneck in this environment is the ~45MB/s axon tunnel
between host and the tunneled devices, so the design minimizes wire
bytes and overlaps host compute / upload / device exec / download in a
chunked pipeline (CHUNKS batch chunks, each spread over all 8 cores):

  host (f32):  act_v0 = relu(v @ Wv.T + bv)     (input FC; f32 kept for
                                                 the final residual add)
               q_s    = q @ Ws[:,OD:].T + bs    (q half of GAT self fc)
               P      = pos_emb . Wp^T + bp     (PD->2H projection:
                                                 128MB -> 32MB bf16)
  wire up:     act_v0 as fp8-e4m3 (8MB), [q_s | P] as bf16 (32MB)
  device:      2 steps x 2 dirs of graph self attention (all GEMMs,
               softmax with geometric log-bias), returns S = rel1+rel2
  wire down:   S as bf16 (16MB), overlapped with uploads (full duplex)
  host (f32):  out = act_v0 + S  (fused into the per-shard fetch)

Exact algebraic simplifications (no approximation):
  - bk (key bias) and the Wb/bb scalar add a softmax-constant per row ->
    dropped entirely.
  - 1/sqrt(DG) folded into Wq/bq; bs folded into q_s.
  - q mask is all-true for this model family (relu rows never all-zero,
    and act_v only grows across steps), so q_exp == q.
  - logits are bounded (|aff|<~3, log-bias in [-13.8, 0]), so softmax
    needs no max-subtraction before exp.

Accuracy: tolerance is 2e-2; measured end-to-end rel err 1.25e-2 (bf16
everywhere + fp8 act_v on the attention path only; fp8 for the position
projection fails - log() amplifies quantization near zero and the
pos-biased softmax is concentrated enough that near-ties flip).

Caching across calls: the Bass program + jitted shard_map runner build
once; weights upload once (keyed on checksums); the donated output
scratch buffers are recycled from the previous call's outputs; host
staging buffers (GEMM outputs, wire-format arrays) persist across calls
to avoid alloc/page-fault overhead (safe: kernel() joins the pipeline
before returning).

Steady-state per-chunk dispatch+exec through the tunnel is ~72ms, almost
all RPC round-trip — the NEFF itself is sub-millisecond, so device-side
tiling is not a lever here.

Hardware gotcha found via bisection: matmuls whose operands sit at SBUF
partition offset 64 crash the exec unit (NRT_EXEC_UNIT_UNRECOVERABLE),
so all per-head operands are laid out at partition 0 via per-head PE
transposes.
"""

import sys
import threading
import concurrent.futures as _cf

import numpy as np

for _p in ('/opt/trn_rl_repo', '/root/.axon_site/_ro/trn_rl_repo'):
    if _p not in sys.path:
        sys.path.append(_p)

import ml_dtypes

BF16 = ml_dtypes.bfloat16
F8 = ml_dtypes.float8_e4m3

# dims (hardcoded per problem spec)
B, N, NG, H = 64, 128, 64, 16
VD, QD, OD, PD = 2048, 1024, 1024, 64
DG = OD // H
DIRS, STEPS = 2, 2
import os as _os
EPS = 1e-6
NCORES = 8
CHUNKS = int(_os.environ.get('K_CHUNKS', '2'))  # pipeline chunks per call
SH = B // NCORES // CHUNKS  # samples per core per chunk
BC = B // CHUNKS          # batch samples per chunk
KT = OD // 128            # 8 contraction tiles of 128

_state = {}
_lock = threading.Lock()


# --------------------------------------------------------------------------
# device program
# --------------------------------------------------------------------------

def _emit(nc, tc, bass, mybir, make_identity, sh, av8, x,
          wsv, wq, wk, wo, bq, bo, outS):
    dt = mybir.dt
    f32 = dt.float32
    b16 = dt.bfloat16
    AF = mybir.ActivationFunctionType
    ALU = mybir.AluOpType
    ts = bass.ts

    with tc.tile_pool(name="wpool", bufs=1) as wpool, \
         tc.tile_pool(name="cpool", bufs=1) as cpool, \
         tc.tile_pool(name="apool", bufs=2) as apool, \
         tc.tile_pool(name="a1pool", bufs=1) as a1pool, \
         tc.tile_pool(name="ppool", bufs=3, space="PSUM") as ppool:

        # ---- weights, resident in SBUF for the whole kernel ----
        wsv_sb = wpool.tile([128, KT, OD], b16, tag="wsv")
        nc.sync.dma_start(wsv_sb, wsv.rearrange("(kt p) od -> p kt od", p=128))
        wq_sb = wpool.tile([128, DIRS, KT, OD], b16, tag="wq")
        nc.sync.dma_start(wq_sb, wq.rearrange("(d kt p) od -> p d kt od",
                                              d=DIRS, p=128))
        wk_sb = wpool.tile([128, DIRS, KT, OD], b16, tag="wk")
        nc.sync.dma_start(wk_sb, wk.rearrange("(d kt p) od -> p d kt od",
                                              d=DIRS, p=128))
        wo_sb = wpool.tile([128, DIRS, KT, OD], b16, tag="wo")
        nc.sync.dma_start(wo_sb, wo.rearrange("(d kt p) od -> p d kt od",
                                              d=DIRS, p=128))
        bq_sb = cpool.tile([1, DIRS * OD], b16, tag="bq")
        nc.sync.dma_start(bq_sb, bq)
        bo_sb = cpool.tile([128, OD], b16, tag="bo")
        nc.sync.dma_start(bo_sb, bo[0, :].partition_broadcast(128))
        ones_sb = cpool.tile([1, 128], b16, tag="ones")
        nc.vector.memset(ones_sb, 1.0)
        ident = cpool.tile([128, 128], b16, tag="ident")
        make_identity(nc, ident)

        # av8: fp8 [s*n, od]; x per-sample block: row 0 q_s,
        # rows 1:257 pp ([n, (d h m)] as 2 rows of 1024 per n)
        RB = 1 + 2 * N
        av_r = av8.rearrange("(s n) od -> s n od", s=sh)
        x_r = x.rearrange("(s r) od -> s r od", s=sh, r=RB)
        out_r = outS.rearrange("(s n) od -> s n od", s=sh)

        for s in range(sh):
            av8_sb = apool.tile([128, OD], dt.float8e4, tag="av8")  # [n, od]
            nc.sync.dma_start(av8_sb, av_r[s])
            av_sb = apool.tile([128, OD], b16, tag="av")
            nc.vector.tensor_copy(av_sb, av8_sb)
            avT_ps = ppool.tile([128, OD], b16, tag="work")
            for kt in range(KT):
                nc.tensor.transpose(avT_ps[:, ts(kt, 128)],
                                    av_sb[:, ts(kt, 128)], ident)
            avT = apool.tile([128, OD], b16, tag="avT")        # [od, n]
            nc.vector.tensor_copy(avT, avT_ps)
            pp_sb = apool.tile([128, DIRS * H * NG], b16, tag="pp")
            nc.sync.dma_start(
                pp_sb, x_r[s, 1:RB, :].rearrange("(n h) od -> n (h od)",
                                                 h=2))
            qs_sb = apool.tile([1, OD], b16, tag="qs")
            nc.sync.dma_start(qs_sb, x_r[s, 0:1, :])

            xT = avT
            rel_tiles = []
            for t in range(STEPS):
                # ---- self_feat = x @ Wsv.T + q_s  (rank-1 bias) ----
                sf_ps = ppool.tile([128, OD], f32, tag="work")
                for half in range(2):
                    sl = slice(half * 512, (half + 1) * 512)
                    for kt in range(KT):
                        nc.tensor.matmul(sf_ps[:, sl], lhsT=xT[:, ts(kt, 128)],
                                         rhs=wsv_sb[:, kt, sl],
                                         start=(kt == 0), stop=False)
                    nc.tensor.matmul(sf_ps[:, sl], lhsT=ones_sb,
                                     rhs=qs_sb[:, sl], start=False, stop=True)
                sf_bf = apool.tile([128, OD], b16, tag="sf")
                nc.vector.tensor_copy(sf_bf, sf_ps)
                # ---- sfT (transposed self_feat) ----
                sfT_ps = ppool.tile([128, OD], b16, tag="work")
                for kt in range(KT):
                    nc.tensor.transpose(sfT_ps[:, ts(kt, 128)],
                                        sf_bf[:, ts(kt, 128)], ident)
                sfT = apool.tile([128, OD], b16, tag="sfT")
                nc.vector.tensor_copy(sfT, sfT_ps)

                acc_sb = a1pool.tile([128, OD], f32, tag="acc_sb")
                for d in range(DIRS):
                    # ---- qh = sf @ Wq[d].T + bq[d]  -> qhT ----
                    qh_ps = ppool.tile([128, OD], f32, tag="work")
                    for half in range(2):
                        sl = slice(half * 512, (half + 1) * 512)
                        for kt in range(KT):
                            nc.tensor.matmul(qh_ps[:, sl],
                                             lhsT=sfT[:, ts(kt, 128)],
                                             rhs=wq_sb[:, d, kt, sl],
                                             start=(kt == 0), stop=False)
                        nc.tensor.matmul(
                            qh_ps[:, sl], lhsT=ones_sb,
                            rhs=bq_sb[:, d * OD + half * 512:
                                      d * OD + (half + 1) * 512],
                            start=False, stop=True)
                    qh_bf = apool.tile([128, OD], b16, tag="qh")
                    nc.vector.tensor_copy(qh_bf, qh_ps)
                    # per-head transpose so operands stay at partition 0
                    # (matmuls with partition-base-64 operands hang the PE)
                    qhT_ps_a = ppool.tile([64, H * 64], b16, tag="work")
                    qhT_ps_b = ppool.tile([64, H * 64], b16, tag="work")
                    for h in range(H):
                        tgt = qhT_ps_a if h < 8 else qhT_ps_b
                        nc.tensor.transpose(tgt[:, ts(h % 8, 128)],
                                            qh_bf[:, ts(h, 64)], ident)
                    qhT = apool.tile([64, H * 128], b16, tag="qhT")
                    nc.vector.tensor_copy(qhT[:, :H * 64], qhT_ps_a)
                    nc.vector.tensor_copy(qhT[:, H * 64:], qhT_ps_b)

                    # ---- kh = kv @ Wk[d].T  -> khT ----
                    kh_ps = ppool.tile([64, OD], f32, tag="work")
                    for half in range(2):
                        sl = slice(half * 512, (half + 1) * 512)
                        for kt in range(KT):
                            nc.tensor.matmul(
                                kh_ps[:, sl],
                                lhsT=sfT[:, kt * 128:kt * 128 + 64],
                                rhs=wk_sb[:, d, kt, sl],
                                start=(kt == 0), stop=(kt == KT - 1))
                    kh_bf = apool.tile([64, OD], b16, tag="kh")
                    nc.vector.tensor_copy(kh_bf, kh_ps)
                    khT_ps = ppool.tile([64, H * 64], b16, tag="work")
                    for h in range(H):
                        nc.tensor.transpose(khT_ps[:, ts(h, 64)],
                                            kh_bf[:, ts(h, 64)],
                                            ident[:64, :64])
                    khT = apool.tile([64, H * 64], b16, tag="khT")
                    nc.vector.tensor_copy(khT, khT_ps)

                    # ---- kvW = kv @ WoT[d] ----
                    kvw_ps = ppool.tile([64, OD], f32, tag="work")
                    for half in range(2):
                        sl = slice(half * 512, (half + 1) * 512)
                        for kt in range(KT):
                            nc.tensor.matmul(
                                kvw_ps[:, sl],
                                lhsT=sfT[:, kt * 128:kt * 128 + 64],
                                rhs=wo_sb[:, d, kt, sl],
                                start=(kt == 0), stop=(kt == KT - 1))
                    kvw = apool.tile([64, OD], b16, tag="kvw")
                    nc.vector.tensor_copy(kvw, kvw_ps)

                    # ---- aff[n, (h,m)] = qh_h @ kh_h.T (pre-scaled) ----
                    aff_ps = ppool.tile([128, OD], f32, tag="work")
                    for h in range(H):
                        nc.tensor.matmul(aff_ps[:, ts(h, 64)],
                                         lhsT=qhT[:, ts(h, 128)],
                                         rhs=khT[:, ts(h, 64)],
                                         start=True, stop=True)

                    # ---- logits = aff + ln(max(P_d, eps)); softmax ----
                    # logits are bounded (|aff|<~3, lp in [-13.8, 0]) so no
                    # max-subtraction is needed before exp.
                    lg = a1pool.tile([128, H * NG], f32, tag="lg")
                    nc.vector.tensor_scalar_max(
                        lg, pp_sb[:, d * H * NG:(d + 1) * H * NG], EPS)
                    nc.scalar.activation(lg, lg, AF.Ln)
                    nc.vector.tensor_tensor(lg, aff_ps, lg, op=ALU.add)
                    nc.scalar.activation(lg, lg, AF.Exp)
                    lg3 = lg.rearrange("p (h m) -> p h m", h=H)
                    sums = apool.tile([128, H], f32, tag="sums")
                    nc.vector.reduce_sum(sums, lg3, axis=mybir.AxisListType.X)
                    rsum = apool.tile([128, H], f32, tag="rsum")
                    nc.vector.reciprocal(rsum, sums)
                    att = apool.tile([128, H * NG], b16, tag="att")
                    for h in range(H):
                        nc.vector.tensor_scalar_mul(att[:, ts(h, 64)],
                                                    lg[:, ts(h, 64)],
                                                    rsum[:, h:h + 1])

                    # ---- attT, out_t = att @ kvW (accumulated over d) ----
                    attT_ps_a = ppool.tile([64, OD], b16, tag="work")
                    attT_ps_b = ppool.tile([64, OD], b16, tag="work")
                    for h in range(H):
                        tgt = attT_ps_a if h < 8 else attT_ps_b
                        nc.tensor.transpose(tgt[:, ts(h % 8, 128)],
                                            att[:, ts(h, 64)], ident)
                    attT = a1pool.tile([64, H * 128], b16, tag="attT")
                    nc.vector.tensor_copy(attT[:, :H * 64], attT_ps_a)
                    nc.vector.tensor_copy(attT[:, H * 64:], attT_ps_b)
                    ot_ps = ppool.tile([128, OD], f32, tag="work")
                    for h in range(H):
                        nc.tensor.matmul(ot_ps[:, ts(h, 64)],
                                         lhsT=attT[:, ts(h, 128)],
                                         rhs=kvw[:, ts(h, 64)],
                                         start=True, stop=True)
                    if d == 0:
                        nc.vector.tensor_tensor(acc_sb, ot_ps, sf_bf,
                                                op=ALU.add)
                    else:
                        nc.vector.tensor_tensor(acc_sb, ot_ps, acc_sb,
                                                op=ALU.add)

                # ---- rel = relu(self_feat + attn0 + attn1 + bout) ----
                tmp = a1pool.tile([128, OD], f32, tag="lg")
                nc.vector.tensor_tensor(tmp, acc_sb, bo_sb, op=ALU.add)
                rel_bf = apool.tile([128, OD], b16, tag=f"rel{t}")
                nc.scalar.activation(rel_bf, tmp, AF.Relu)
                rel_tiles.append(rel_bf)

                if t == 0:
                    relT_ps = ppool.tile([128, OD], b16, tag="work")
                    for kt in range(KT):
                        nc.tensor.transpose(relT_ps[:, ts(kt, 128)],
                                            rel_bf[:, ts(kt, 128)], ident)
                    xT2 = apool.tile([128, OD], b16, tag="xT2")
                    nc.vector.tensor_tensor(xT2, relT_ps, avT, op=ALU.add)
                    xT = xT2

            outb = apool.tile([128, OD], b16, tag="outb")
            nc.vector.tensor_tensor(outb, rel_tiles[0], rel_tiles[1],
                                    op=ALU.add)
            nc.sync.dma_start(out_r[s], outb)


def _build_program(sh):
    import concourse.bass as bass
    import concourse.tile as tile
    from concourse import bacc, mybir
    from concourse.masks import make_identity
    dt = mybir.dt

    nc = bacc.Bacc("TRN2", target_bir_lowering=False, debug=False,
                   num_devices=NCORES)
    av8 = nc.dram_tensor("av8", [sh * N, OD], dt.float8e4,
                         kind="ExternalInput")
    x = nc.dram_tensor("x", [sh * (2 * N + 1), OD], dt.bfloat16,
                       kind="ExternalInput")
    wsv = nc.dram_tensor("wsv", [OD, OD], dt.bfloat16, kind="ExternalInput")
    wq = nc.dram_tensor("wq", [DIRS * OD, OD], dt.bfloat16,
                        kind="ExternalInput")
    wk = nc.dram_tensor("wk", [DIRS * OD, OD], dt.bfloat16,
                        kind="ExternalInput")
    wo = nc.dram_tensor("wo", [DIRS * OD, OD], dt.bfloat16,
                        kind="ExternalInput")
    bq = nc.dram_tensor("bq", [1, DIRS * OD], dt.bfloat16,
                        kind="ExternalInput")
    bo = nc.dram_tensor("bo", [1, OD], dt.bfloat16, kind="ExternalInput")
    outS = nc.dram_tensor("outS", [sh * N, OD], dt.bfloat16,
                          kind="ExternalOutput")

    with tile.TileContext(nc) as tc:
        _emit(nc, tc, bass, mybir, make_identity, sh,
              av8.ap(), x.ap(), wsv.ap(), wq.ap(), wk.ap(),
              wo.ap(), bq.ap(), bo.ap(), outS.ap())
    nc.compile()
    return nc


# --------------------------------------------------------------------------
# runner (jit + shard_map over 8 cores, cached across calls)
# --------------------------------------------------------------------------

def _build_runner(nc):
    import jax
    from jax.experimental.shard_map import shard_map
    from jax.sharding import Mesh, PartitionSpec, NamedSharding
    from concourse import bass2jax, mybir

    bass2jax.install_neuronx_cc_hook()

    in_names, out_names, out_avals = [], [], []
    for alloc in nc.m.functions[0].allocations:
        if not isinstance(alloc, mybir.MemoryLocationSet):
            continue
        name = alloc.memorylocations[0].name
        if alloc.kind == "ExternalInput":
            in_names.append(name)
        elif alloc.kind == "ExternalOutput":
            out_names.append(name)
            shape = tuple(alloc.tensor_shape)
            dtype = mybir.dt.np(alloc.dtype)
            out_avals.append(jax.core.ShapedArray(shape, dtype))
    n_params = len(in_names)
    n_outs = len(out_names)
    all_names = tuple(in_names + out_names)

    def _body(*args):
        outs = bass2jax._bass_exec_p.bind(
            *args,
            out_avals=tuple(out_avals),
            in_names=all_names,
            out_names=tuple(out_names),
            lowering_input_output_aliases=(),
            sim_require_finite=True,
            sim_require_nnan=True,
            nc=nc,
        )
        return tuple(outs)

    devices = jax.devices()[:NCORES]
    mesh = Mesh(np.asarray(devices), ("core",))
    in_specs = (PartitionSpec("core"),) * (n_params + n_outs)
    out_specs = (PartitionSpec("core"),) * n_outs
    donate = tuple(range(n_params, n_params + n_outs))
    fn = jax.jit(
        shard_map(_body, mesh=mesh, in_specs=in_specs, out_specs=out_specs,
                  check_rep=False),
        donate_argnums=donate, keep_unused=True)
    sharding = NamedSharding(mesh, PartitionSpec("core"))
    return fn, in_names, out_names, sharding


# per-call batch chunk sizes (samples), each divisible by NCORES.
# Symmetric halves measured best: asymmetric splits pay more in delayed
# pipe start / CPU contention than they save on the download tail.
CHUNK_SIZES = [int(t) for t in _os.environ.get('K_SIZES', '32,32').split(',')]
assert sum(CHUNK_SIZES) == B and all(s % NCORES == 0 for s in CHUNK_SIZES)


def _ensure_built():
    with _lock:
        if 'fns' in _state:
            return
        import jax
        fns = {}
        for sh in sorted({s // NCORES for s in CHUNK_SIZES}):
            nc = _build_program(sh)
            fn, in_names, out_names, sharding = _build_runner(nc)
            fns[sh] = (fn, in_names)
            _state['sharding'] = sharding
        _state['fns'] = fns
        _state['jax'] = jax
        _state['devices'] = jax.devices()[:NCORES]
        _state['pool'] = _cf.ThreadPoolExecutor(max_workers=1)
        _state['runner'] = _cf.ThreadPoolExecutor(max_workers=1)
        _state['tp'] = _cf.ThreadPoolExecutor(max_workers=NCORES)
        _state['scratch'] = [None] * len(CHUNK_SIZES)


def _put_sharded(arr):
    """Upload a [R, C] array row-sharded over the 8 cores, with the
    per-device transfers issued in parallel (hides per-device latency)."""
    st = _state
    jax = st['jax']
    devs = st['devices']
    rows = arr.shape[0] // NCORES
    futs = [st['tp'].submit(jax.device_put, arr[i * rows:(i + 1) * rows],
                            devs[i]) for i in range(NCORES)]
    shards = [f.result() for f in futs]
    return jax.make_array_from_single_device_arrays(
        arr.shape, st['sharding'], shards)


def _fetch_add_sharded(out, act, dst):
    """Fetch a sharded device array (bf16 [R, OD]) with parallel per-shard
    d2h transfers, adding act (f32 [R, OD]) into dst (f32 [R, OD]) per
    shard as it arrives."""
    st = _state
    shards = sorted(out.addressable_shards, key=lambda s: s.index[0].start)
    rows = act.shape[0] // NCORES

    def get(i):
        sl = slice(i * rows, (i + 1) * rows)
        S = np.asarray(shards[i].data)
        np.add(act[sl], S, out=dst[sl], dtype=np.float32, casting='unsafe')
    list(st['tp'].map(get, range(NCORES)))


# --------------------------------------------------------------------------
# host pre/post processing
# --------------------------------------------------------------------------

def _prep_weights(inputs):
    Ws = np.asarray(inputs['Ws'], np.float32)
    bs = np.asarray(inputs['bs'], np.float32)
    Wq = np.asarray(inputs['Wq'], np.float32)
    bq = np.asarray(inputs['bq'], np.float32)
    Wk = np.asarray(inputs['Wk'], np.float32)
    Wout = np.asarray(inputs['Wout'], np.float32)
    bout = np.asarray(inputs['bout'], np.float32)
    scale = 1.0 / np.sqrt(np.float32(DG))
    w = {
        'wsv': np.ascontiguousarray(Ws[:, :OD].T).astype(BF16),
        'wq': np.concatenate([np.ascontiguousarray((Wq[d] * scale).T)
                              for d in range(DIRS)], 0).astype(BF16),
        'wk': np.concatenate([np.ascontiguousarray(Wk[d].T)
                              for d in range(DIRS)], 0).astype(BF16),
        'wo': np.concatenate([Wout[d].transpose(2, 0, 1).reshape(OD, OD)
                              for d in range(DIRS)], 0).astype(BF16),
        'bq': (bq.reshape(1, DIRS * OD) * scale).astype(BF16),
        'bo': bout.sum(0).reshape(1, OD).astype(BF16),
    }
    return w, Ws[:, OD:], bs


def kernel(**inputs) -> np.ndarray:
    _ensure_built()
    st = _state
    jax = st['jax']
    pool = st['pool']
    put = lambda arr: jax.device_put(arr, st['sharding'])

    v = np.asarray(inputs['v'], np.float32)
    pos = np.asarray(inputs['position_embedding'], np.float32)
    q = np.asarray(inputs['q'], np.float32)
    Wv = np.asarray(inputs['Wv'], np.float32)
    bv = np.asarray(inputs['bv'], np.float32)
    Wp = np.asarray(inputs['Wp'], np.float32)
    bp = np.asarray(inputs['bp'], np.float32)

    futs = {}

    # ---- weights: upload once, cache on device ----
    # change-detection key: f32 pairwise sums (memory-bound, ~20ms total)
    wkey = (float(np.asarray(inputs['Ws']).sum()),
            float(np.asarray(inputs['Wq']).sum()),
            float(np.asarray(inputs['Wk']).sum()),
            float(np.asarray(inputs['Wout']).sum()))
    if st.get('wkey') != wkey:
        w, Wsq, bs = _prep_weights(inputs)
        st['Wsq'] = Wsq
        st['bs'] = bs
        for name, arr in w.items():
            futs[name] = pool.submit(put, np.concatenate([arr] * NCORES, 0))
        st['wkey'] = wkey

    # resolve weight device arrays (first call only)
    wdev = st.setdefault('wdev', {})
    for name in ('wsv', 'wq', 'wk', 'wo', 'bq', 'bo'):
        if name in futs:
            wdev[name] = futs[name].result()

    # ---- q_s = q @ Ws_q.T + bs  (whole batch, tiny) ----
    qs_b = (q @ st['Wsq'].T + st['bs']).astype(BF16)

    # ---- chunked pipeline: host compute -> upload -> exec -> download ----
    Wp_all = Wp.reshape(DIRS * H, PD).T
    bp_row = bp.reshape(1, DIRS * H)
    nchunks = len(CHUNK_SIZES)
    result = np.empty((B, N, OD), np.float32)
    act_chunks = [None] * nchunks
    cfuts = [None] * nchunks

    RB = 2 * N + 1
    result2 = result.reshape(B * N, OD)

    def run_chunk(c, bc, fn, in_names, r0, fav, fx):
        args = {'av8': fav.result(), 'x': fx.result(), **wdev}
        scratch = st['scratch'][c]
        if scratch is None:
            scratch = jax.device_put(np.zeros((bc * N, OD), BF16),
                                     st['sharding'])
        (out,) = fn(*[args[n] for n in in_names], scratch)
        _fetch_add_sharded(out, act_chunks[c], result2[r0:r0 + bc * N])
        st['scratch'][c] = out

    # persistent host buffers (avoid per-call alloc + page faults); safe
    # to reuse: kernel() joins all pipeline work before returning.
    bufs = st.setdefault('hostbufs', {})

    def hbuf(key, shape, dtype):
        b = bufs.get(key)
        if b is None or b.shape != tuple(shape):
            b = np.empty(shape, dtype)
            bufs[key] = b
        return b

    b0 = 0
    for c, bc in enumerate(CHUNK_SIZES):
        bsl = slice(b0, b0 + bc)
        fn, in_names = st['fns'][bc // NCORES]
        X = hbuf(('X', c), (bc, RB, OD), BF16)
        # position projection chunk: [bc,N,NG,PD] -> [bc,N,(d,h),NG] bf16
        P = hbuf(('P', c), (bc * N * NG, DIRS * H), np.float32)
        np.matmul(pos[bsl].reshape(-1, PD), Wp_all, out=P)
        P += bp_row
        # direct strided cast-assign into the X view (single pass)
        X[:, 1:, :].reshape(bc, N, DIRS * H, NG)[...] = \
            P.reshape(bc, N, NG, DIRS * H).transpose(0, 1, 3, 2)
        X[:, 0, :] = qs_b[bsl]
        fx = pool.submit(_put_sharded, X.reshape(bc * RB, OD))
        # v transform chunk: act_v0 = relu(v @ Wv.T + bv), fp8 on the wire
        a = hbuf(('a', c), (bc * N, OD), np.float32)
        np.matmul(v[bsl].reshape(-1, VD), Wv.T, out=a)
        a += bv
        np.maximum(a, 0, out=a)                # [bc*N, OD] f32
        act_chunks[c] = a
        a8 = hbuf(('a8', c), (bc * N, OD), F8)
        a8[...] = a
        fav = pool.submit(_put_sharded, a8)
        cfuts[c] = st['runner'].submit(run_chunk, c, bc, fn, in_names,
                                       b0 * N, fav, fx)
        b0 += bc

    for c in range(nchunks):
        cfuts[c].result()
    return result


if __name__ == '__main__':
    rng = np.random.default_rng(0)
    ins = {
        'v': rng.standard_normal((B, N, VD)).astype(np.float32),
        'position_embedding': rng.random((B, N, NG, PD)).astype(np.float32),
        'q': rng.standard_normal((B, QD)).astype(np.float32),
        'Wv': 0.02 * rng.standard_normal((OD, VD)).astype(np.float32),
        'bv': np.zeros(OD, np.float32),
        'Ws': 0.02 * rng.standard_normal((OD, OD + QD)).astype(np.float32),
        'bs': np.zeros(OD, np.float32),
        'Wb': 0.02 * rng.standard_normal((1, 1)).astype(np.float32),
        'bb': np.zeros(1, np.float32),
        'Wq': 0.02 * rng.standard_normal((DIRS, OD, OD)).astype(np.float32),
        'bq': np.zeros((DIRS, OD), np.float32),
        'Wk': 0.02 * rng.standard_normal((DIRS, OD, OD)).astype(np.float32),
        'bk': np.zeros((DIRS, OD), np.float32),
        'Wp': 0.02 * rng.standard_normal((DIRS, H, PD)).astype(np.float32),
        'bp': np.zeros((DIRS, H), np.float32),
        'Wout': 0.02 * rng.standard_normal((DIRS, H, DG, OD)).astype(np.float32),
        'bout': np.zeros((DIRS, OD), np.float32),
    }
    out = kernel(**ins)
    print('kernel output', out.shape, out.dtype, float(np.abs(out).mean()))


# revision 17
# speedup vs baseline: 1.0628x; 1.0628x over previous
"""ActionRelationEncoder Bass kernel for 8 Trainium2 NeuronCores.

Data-parallel over batch (B=64), weights replicated on every core. The
wall-clock bottleneck in this environment is the ~45MB/s axon tunnel
between host and the tunneled devices, so the design minimizes wire
bytes and overlaps host compute / upload / device exec / download in a
chunked pipeline (CHUNKS batch chunks, each spread over all 8 cores):

  host (f32):  act_v0 = relu(v @ Wv.T + bv)     (input FC; f32 kept for
                                                 the final residual add)
               q_s    = q @ Ws[:,OD:].T + bs    (q half of GAT self fc)
               P      = pos_emb . Wp^T + bp     (PD->2H projection:
                                                 128MB -> 32MB bf16)
  wire up:     act_v0 as fp8-e4m3 (8MB), [q_s | P] as bf16 (32MB)
  device:      2 steps x 2 dirs of graph self attention (all GEMMs,
               softmax with geometric log-bias), returns S = rel1+rel2
  wire down:   S as bf16 (16MB), overlapped with uploads (full duplex)
  host (f32):  out = act_v0 + S  (fused into the per-shard fetch)

Exact algebraic simplifications (no approximation):
  - bk (key bias) and the Wb/bb scalar add a softmax-constant per row ->
    dropped entirely.
  - 1/sqrt(DG) folded into Wq/bq; bs folded into q_s.
  - q mask is all-true for this model family (relu rows never all-zero,
    and act_v only grows across steps), so q_exp == q.
  - logits are bounded (|aff|<~3, log-bias in [-13.8, 0]), so softmax
    needs no max-subtraction before exp.

Accuracy: tolerance is 2e-2; measured end-to-end rel err 1.25e-2 (bf16
everywhere + fp8 act_v on the attention path only; fp8 for the position
projection fails - log() amplifies quantization near zero and the
pos-biased softmax is concentrated enough that near-ties flip).

Caching across calls: the Bass program + jitted shard_map runner build
once; weights upload once (keyed on checksums); the donated output
scratch buffers are recycled from the previous call's outputs; host
staging buffers (GEMM outputs, wire-format arrays) persist across calls
to avoid alloc/page-fault overhead (safe: kernel() joins the pipeline
before returning).

Steady-state per-chunk dispatch+exec through the tunnel is ~72ms, almost
all RPC round-trip — the NEFF itself is sub-millisecond, so device-side
tiling is not a lever here.

Hardware gotcha found via bisection: matmuls whose operands sit at SBUF
partition offset 64 crash the exec unit (NRT_EXEC_UNIT_UNRECOVERABLE),
so all per-head operands are laid out at partition 0 via per-head PE
transposes.
"""

import sys
import threading
import concurrent.futures as _cf

import numpy as np

for _p in ('/opt/trn_rl_repo', '/root/.axon_site/_ro/trn_rl_repo'):
    if _p not in sys.path:
        sys.path.append(_p)

import ml_dtypes

BF16 = ml_dtypes.bfloat16
F8 = ml_dtypes.float8_e4m3

# dims (hardcoded per problem spec)
B, N, NG, H = 64, 128, 64, 16
VD, QD, OD, PD = 2048, 1024, 1024, 64
DG = OD // H
DIRS, STEPS = 2, 2
import os as _os
EPS = 1e-6
NCORES = 8
CHUNKS = int(_os.environ.get('K_CHUNKS', '2'))  # pipeline chunks per call
SH = B // NCORES // CHUNKS  # samples per core per chunk
BC = B // CHUNKS          # batch samples per chunk
KT = OD // 128            # 8 contraction tiles of 128

_state = {}
_lock = threading.Lock()


# --------------------------------------------------------------------------
# device program
# --------------------------------------------------------------------------

def _emit(nc, tc, bass, mybir, make_identity, sh, av8, x,
          wsv, wq, wk, wo, bq, bo, outS, scl):
    dt = mybir.dt
    f32 = dt.float32
    b16 = dt.bfloat16
    AF = mybir.ActivationFunctionType
    ALU = mybir.AluOpType
    ts = bass.ts

    with tc.tile_pool(name="wpool", bufs=1) as wpool, \
         tc.tile_pool(name="cpool", bufs=1) as cpool, \
         tc.tile_pool(name="apool", bufs=2) as apool, \
         tc.tile_pool(name="a1pool", bufs=1) as a1pool, \
         tc.tile_pool(name="ppool", bufs=3, space="PSUM") as ppool:

        # ---- weights, resident in SBUF for the whole kernel ----
        wsv_sb = wpool.tile([128, KT, OD], b16, tag="wsv")
        nc.sync.dma_start(wsv_sb, wsv.rearrange("(kt p) od -> p kt od", p=128))
        wq_sb = wpool.tile([128, DIRS, KT, OD], b16, tag="wq")
        nc.sync.dma_start(wq_sb, wq.rearrange("(d kt p) od -> p d kt od",
                                              d=DIRS, p=128))
        wk_sb = wpool.tile([128, DIRS, KT, OD], b16, tag="wk")
        nc.sync.dma_start(wk_sb, wk.rearrange("(d kt p) od -> p d kt od",
                                              d=DIRS, p=128))
        wo_sb = wpool.tile([128, DIRS, KT, OD], b16, tag="wo")
        nc.sync.dma_start(wo_sb, wo.rearrange("(d kt p) od -> p d kt od",
                                              d=DIRS, p=128))
        bq_sb = cpool.tile([1, DIRS * OD], b16, tag="bq")
        nc.sync.dma_start(bq_sb, bq)
        bo_sb = cpool.tile([128, OD], b16, tag="bo")
        nc.sync.dma_start(bo_sb, bo[0, :].partition_broadcast(128))
        ones_sb = cpool.tile([1, 128], b16, tag="ones")
        nc.vector.memset(ones_sb, 1.0)
        ident = cpool.tile([128, 128], b16, tag="ident")
        make_identity(nc, ident)

        # av8: fp8 [s*n, od]; x per-sample block: row 0 q_s,
        # rows 1:257 pp ([n, (d h m)] as 2 rows of 1024 per n)
        RB = 1 + 2 * N
        av_r = av8.rearrange("(s n) od -> s n od", s=sh)
        x_r = x.rearrange("(s r) od -> s r od", s=sh, r=RB)
        out_r = outS.rearrange("(s n) od -> s n od", s=sh)
        scl_r = scl.rearrange("(s n) one -> s n one", s=sh)

        for s in range(sh):
            av8_sb = apool.tile([128, OD], dt.float8e4, tag="av8")  # [n, od]
            nc.sync.dma_start(av8_sb, av_r[s])
            av_sb = apool.tile([128, OD], b16, tag="av")
            nc.vector.tensor_copy(av_sb, av8_sb)
            avT_ps = ppool.tile([128, OD], b16, tag="work")
            for kt in range(KT):
                nc.tensor.transpose(avT_ps[:, ts(kt, 128)],
                                    av_sb[:, ts(kt, 128)], ident)
            avT = apool.tile([128, OD], b16, tag="avT")        # [od, n]
            nc.vector.tensor_copy(avT, avT_ps)
            pp_sb = apool.tile([128, DIRS * H * NG], b16, tag="pp")
            nc.sync.dma_start(
                pp_sb, x_r[s, 1:RB, :].rearrange("(n h) od -> n (h od)",
                                                 h=2))
            qs_sb = apool.tile([1, OD], b16, tag="qs")
            nc.sync.dma_start(qs_sb, x_r[s, 0:1, :])

            xT = avT
            rel_tiles = []
            for t in range(STEPS):
                # ---- self_feat = x @ Wsv.T + q_s  (rank-1 bias) ----
                sf_ps = ppool.tile([128, OD], f32, tag="work")
                for half in range(2):
                    sl = slice(half * 512, (half + 1) * 512)
                    for kt in range(KT):
                        nc.tensor.matmul(sf_ps[:, sl], lhsT=xT[:, ts(kt, 128)],
                                         rhs=wsv_sb[:, kt, sl],
                                         start=(kt == 0), stop=False)
                    nc.tensor.matmul(sf_ps[:, sl], lhsT=ones_sb,
                                     rhs=qs_sb[:, sl], start=False, stop=True)
                sf_bf = apool.tile([128, OD], b16, tag="sf")
                nc.vector.tensor_copy(sf_bf, sf_ps)
                # ---- sfT (transposed self_feat) ----
                sfT_ps = ppool.tile([128, OD], b16, tag="work")
                for kt in range(KT):
                    nc.tensor.transpose(sfT_ps[:, ts(kt, 128)],
                                        sf_bf[:, ts(kt, 128)], ident)
                sfT = apool.tile([128, OD], b16, tag="sfT")
                nc.vector.tensor_copy(sfT, sfT_ps)

                acc_sb = a1pool.tile([128, OD], f32, tag="acc_sb")
                for d in range(DIRS):
                    # ---- qh = sf @ Wq[d].T + bq[d]  -> qhT ----
                    qh_ps = ppool.tile([128, OD], f32, tag="work")
                    for half in range(2):
                        sl = slice(half * 512, (half + 1) * 512)
                        for kt in range(KT):
                            nc.tensor.matmul(qh_ps[:, sl],
                                             lhsT=sfT[:, ts(kt, 128)],
                                             rhs=wq_sb[:, d, kt, sl],
                                             start=(kt == 0), stop=False)
                        nc.tensor.matmul(
                            qh_ps[:, sl], lhsT=ones_sb,
                            rhs=bq_sb[:, d * OD + half * 512:
                                      d * OD + (half + 1) * 512],
                            start=False, stop=True)
                    qh_bf = apool.tile([128, OD], b16, tag="qh")
                    nc.vector.tensor_copy(qh_bf, qh_ps)
                    # per-head transpose so operands stay at partition 0
                    # (matmuls with partition-base-64 operands hang the PE)
                    qhT_ps_a = ppool.tile([64, H * 64], b16, tag="work")
                    qhT_ps_b = ppool.tile([64, H * 64], b16, tag="work")
                    for h in range(H):
                        tgt = qhT_ps_a if h < 8 else qhT_ps_b
                        nc.tensor.transpose(tgt[:, ts(h % 8, 128)],
                                            qh_bf[:, ts(h, 64)], ident)
                    qhT = apool.tile([64, H * 128], b16, tag="qhT")
                    nc.vector.tensor_copy(qhT[:, :H * 64], qhT_ps_a)
                    nc.vector.tensor_copy(qhT[:, H * 64:], qhT_ps_b)

                    # ---- kh = kv @ Wk[d].T  -> khT ----
                    kh_ps = ppool.tile([64, OD], f32, tag="work")
                    for half in range(2):
                        sl = slice(half * 512, (half + 1) * 512)
                        for kt in range(KT):
                            nc.tensor.matmul(
                                kh_ps[:, sl],
                                lhsT=sfT[:, kt * 128:kt * 128 + 64],
                                rhs=wk_sb[:, d, kt, sl],
                                start=(kt == 0), stop=(kt == KT - 1))
                    kh_bf = apool.tile([64, OD], b16, tag="kh")
                    nc.vector.tensor_copy(kh_bf, kh_ps)
                    khT_ps = ppool.tile([64, H * 64], b16, tag="work")
                    for h in range(H):
                        nc.tensor.transpose(khT_ps[:, ts(h, 64)],
                                            kh_bf[:, ts(h, 64)],
                                            ident[:64, :64])
                    khT = apool.tile([64, H * 64], b16, tag="khT")
                    nc.vector.tensor_copy(khT, khT_ps)

                    # ---- kvW = kv @ WoT[d] ----
                    kvw_ps = ppool.tile([64, OD], f32, tag="work")
                    for half in range(2):
                        sl = slice(half * 512, (half + 1) * 512)
                        for kt in range(KT):
                            nc.tensor.matmul(
                                kvw_ps[:, sl],
                                lhsT=sfT[:, kt * 128:kt * 128 + 64],
                                rhs=wo_sb[:, d, kt, sl],
                                start=(kt == 0), stop=(kt == KT - 1))
                    kvw = apool.tile([64, OD], b16, tag="kvw")
                    nc.vector.tensor_copy(kvw, kvw_ps)

                    # ---- aff[n, (h,m)] = qh_h @ kh_h.T (pre-scaled) ----
                    aff_ps = ppool.tile([128, OD], f32, tag="work")
                    for h in range(H):
                        nc.tensor.matmul(aff_ps[:, ts(h, 64)],
                                         lhsT=qhT[:, ts(h, 128)],
                                         rhs=khT[:, ts(h, 64)],
                                         start=True, stop=True)

                    # ---- logits = aff + ln(max(P_d, eps)); softmax ----
                    # logits are bounded (|aff|<~3, lp in [-13.8, 0]) so no
                    # max-subtraction is needed before exp.
                    lg = a1pool.tile([128, H * NG], f32, tag="lg")
                    nc.vector.tensor_scalar_max(
                        lg, pp_sb[:, d * H * NG:(d + 1) * H * NG], EPS)
                    nc.scalar.activation(lg, lg, AF.Ln)
                    nc.vector.tensor_tensor(lg, aff_ps, lg, op=ALU.add)
                    nc.scalar.activation(lg, lg, AF.Exp)
                    lg3 = lg.rearrange("p (h m) -> p h m", h=H)
                    sums = apool.tile([128, H], f32, tag="sums")
                    nc.vector.reduce_sum(sums, lg3, axis=mybir.AxisListType.X)
                    rsum = apool.tile([128, H], f32, tag="rsum")
                    nc.vector.reciprocal(rsum, sums)
                    att = apool.tile([128, H * NG], b16, tag="att")
                    for h in range(H):
                        nc.vector.tensor_scalar_mul(att[:, ts(h, 64)],
                                                    lg[:, ts(h, 64)],
                                                    rsum[:, h:h + 1])

                    # ---- attT, out_t = att @ kvW (accumulated over d) ----
                    attT_ps_a = ppool.tile([64, OD], b16, tag="work")
                    attT_ps_b = ppool.tile([64, OD], b16, tag="work")
                    for h in range(H):
                        tgt = attT_ps_a if h < 8 else attT_ps_b
                        nc.tensor.transpose(tgt[:, ts(h % 8, 128)],
                                            att[:, ts(h, 64)], ident)
                    attT = a1pool.tile([64, H * 128], b16, tag="attT")
                    nc.vector.tensor_copy(attT[:, :H * 64], attT_ps_a)
                    nc.vector.tensor_copy(attT[:, H * 64:], attT_ps_b)
                    ot_ps = ppool.tile([128, OD], f32, tag="work")
                    for h in range(H):
                        nc.tensor.matmul(ot_ps[:, ts(h, 64)],
                                         lhsT=attT[:, ts(h, 128)],
                                         rhs=kvw[:, ts(h, 64)],
                                         start=True, stop=True)
                    if d == 0:
                        nc.vector.tensor_tensor(acc_sb, ot_ps, sf_bf,
                                                op=ALU.add)
                    else:
                        nc.vector.tensor_tensor(acc_sb, ot_ps, acc_sb,
                                                op=ALU.add)

                # ---- rel = relu(self_feat + attn0 + attn1 + bout) ----
                tmp = a1pool.tile([128, OD], f32, tag="lg")
                nc.vector.tensor_tensor(tmp, acc_sb, bo_sb, op=ALU.add)
                rel_bf = apool.tile([128, OD], b16, tag=f"rel{t}")
                nc.scalar.activation(rel_bf, tmp, AF.Relu)
                rel_tiles.append(rel_bf)

                if t == 0:
                    relT_ps = ppool.tile([128, OD], b16, tag="work")
                    for kt in range(KT):
                        nc.tensor.transpose(relT_ps[:, ts(kt, 128)],
                                            rel_bf[:, ts(kt, 128)], ident)
                    xT2 = apool.tile([128, OD], b16, tag="xT2")
                    nc.vector.tensor_tensor(xT2, relT_ps, avT, op=ALU.add)
                    xT = xT2

            outf = a1pool.tile([128, OD], f32, tag="lg")
            nc.vector.tensor_tensor(outf, rel_tiles[0], rel_tiles[1],
                                    op=ALU.add)
            rmax = apool.tile([128, 1], f32, tag="rmax")
            nc.vector.reduce_max(rmax, outf, axis=mybir.AxisListType.X)
            nc.vector.tensor_scalar_max(rmax, rmax, 1e-9)
            rinv = apool.tile([128, 1], f32, tag="rinv")
            nc.vector.reciprocal(rinv, rmax)
            out8 = apool.tile([128, OD], dt.int8, tag="out8")
            nc.vector.tensor_scalar(out8, outf, scalar1=rinv, scalar2=127.0,
                                    op0=ALU.mult, op1=ALU.mult)
            nc.sync.dma_start(out_r[s], out8)
            nc.sync.dma_start(scl_r[s], rmax)


def _build_program(sh):
    import concourse.bass as bass
    import concourse.tile as tile
    from concourse import bacc, mybir
    from concourse.masks import make_identity
    dt = mybir.dt

    nc = bacc.Bacc("TRN2", target_bir_lowering=False, debug=False,
                   num_devices=NCORES)
    av8 = nc.dram_tensor("av8", [sh * N, OD], dt.float8e4,
                         kind="ExternalInput")
    x = nc.dram_tensor("x", [sh * (2 * N + 1), OD], dt.bfloat16,
                       kind="ExternalInput")
    wsv = nc.dram_tensor("wsv", [OD, OD], dt.bfloat16, kind="ExternalInput")
    wq = nc.dram_tensor("wq", [DIRS * OD, OD], dt.bfloat16,
                        kind="ExternalInput")
    wk = nc.dram_tensor("wk", [DIRS * OD, OD], dt.bfloat16,
                        kind="ExternalInput")
    wo = nc.dram_tensor("wo", [DIRS * OD, OD], dt.bfloat16,
                        kind="ExternalInput")
    bq = nc.dram_tensor("bq", [1, DIRS * OD], dt.bfloat16,
                        kind="ExternalInput")
    bo = nc.dram_tensor("bo", [1, OD], dt.bfloat16, kind="ExternalInput")
    outS = nc.dram_tensor("outS", [sh * N, OD], dt.int8,
                          kind="ExternalOutput")
    scl = nc.dram_tensor("scl", [sh * N, 1], dt.float32,
                         kind="ExternalOutput")

    with tile.TileContext(nc) as tc:
        _emit(nc, tc, bass, mybir, make_identity, sh,
              av8.ap(), x.ap(), wsv.ap(), wq.ap(), wk.ap(),
              wo.ap(), bq.ap(), bo.ap(), outS.ap(), scl.ap())
    nc.compile()
    return nc


# --------------------------------------------------------------------------
# runner (jit + shard_map over 8 cores, cached across calls)
# --------------------------------------------------------------------------

def _build_runner(nc):
    import jax
    from jax.experimental.shard_map import shard_map
    from jax.sharding import Mesh, PartitionSpec, NamedSharding
    from concourse import bass2jax, mybir

    bass2jax.install_neuronx_cc_hook()

    in_names, out_names, out_avals = [], [], []
    for alloc in nc.m.functions[0].allocations:
        if not isinstance(alloc, mybir.MemoryLocationSet):
            continue
        name = alloc.memorylocations[0].name
        if alloc.kind == "ExternalInput":
            in_names.append(name)
        elif alloc.kind == "ExternalOutput":
            out_names.append(name)
            shape = tuple(alloc.tensor_shape)
            dtype = mybir.dt.np(alloc.dtype)
            out_avals.append(jax.core.ShapedArray(shape, dtype))
    n_params = len(in_names)
    n_outs = len(out_names)
    all_names = tuple(in_names + out_names)

    def _body(*args):
        outs = bass2jax._bass_exec_p.bind(
            *args,
            out_avals=tuple(out_avals),
            in_names=all_names,
            out_names=tuple(out_names),
            lowering_input_output_aliases=(),
            sim_require_finite=True,
            sim_require_nnan=True,
            nc=nc,
        )
        return tuple(outs)

    devices = jax.devices()[:NCORES]
    mesh = Mesh(np.asarray(devices), ("core",))
    in_specs = (PartitionSpec("core"),) * (n_params + n_outs)
    out_specs = (PartitionSpec("core"),) * n_outs
    donate = tuple(range(n_params, n_params + n_outs))
    fn = jax.jit(
        shard_map(_body, mesh=mesh, in_specs=in_specs, out_specs=out_specs,
                  check_rep=False),
        donate_argnums=donate, keep_unused=True)
    sharding = NamedSharding(mesh, PartitionSpec("core"))
    out_info = [(a.shape, a.dtype) for a in out_avals]
    return fn, in_names, out_info, sharding


# per-call batch chunk sizes (samples), each divisible by NCORES.
# Symmetric halves measured best: asymmetric splits pay more in delayed
# pipe start / CPU contention than they save on the download tail.
CHUNK_SIZES = [int(t) for t in _os.environ.get('K_SIZES', '32,32').split(',')]
assert sum(CHUNK_SIZES) == B and all(s % NCORES == 0 for s in CHUNK_SIZES)


def _ensure_built():
    with _lock:
        if 'fns' in _state:
            return
        import jax
        fns = {}
        for sh in sorted({s // NCORES for s in CHUNK_SIZES}):
            nc = _build_program(sh)
            fn, in_names, out_info, sharding = _build_runner(nc)
            fns[sh] = (fn, in_names, out_info)
            _state['sharding'] = sharding
        _state['fns'] = fns
        _state['jax'] = jax
        _state['devices'] = jax.devices()[:NCORES]
        _state['pool'] = _cf.ThreadPoolExecutor(max_workers=1)
        _state['runner'] = _cf.ThreadPoolExecutor(max_workers=1)
        _state['tp'] = _cf.ThreadPoolExecutor(max_workers=NCORES)
        _state['scratch'] = [None] * len(CHUNK_SIZES)


def _put_sharded(arr):
    """Upload a [R, C] array row-sharded over the 8 cores, with the
    per-device transfers issued in parallel (hides per-device latency)."""
    st = _state
    jax = st['jax']
    devs = st['devices']
    rows = arr.shape[0] // NCORES
    futs = [st['tp'].submit(jax.device_put, arr[i * rows:(i + 1) * rows],
                            devs[i]) for i in range(NCORES)]
    shards = [f.result() for f in futs]
    return jax.make_array_from_single_device_arrays(
        arr.shape, st['sharding'], shards)


def _fetch_add_sharded(out8, scl, act, dst):
    """Fetch the int8 output + per-row f32 scales with parallel per-shard
    d2h transfers, decoding (S = out8 * scl/127) and adding act (f32
    [R, OD]) into dst (f32 [R, OD]) per shard as it arrives."""
    st = _state
    shards8 = sorted(out8.addressable_shards, key=lambda s: s.index[0].start)
    shardsc = sorted(scl.addressable_shards, key=lambda s: s.index[0].start)
    rows = act.shape[0] // NCORES

    def get(i):
        sl = slice(i * rows, (i + 1) * rows)
        S8 = np.asarray(shards8[i].data)
        sc = np.asarray(shardsc[i].data) * np.float32(1.0 / 127.0)
        np.add(act[sl], S8 * sc, out=dst[sl], dtype=np.float32,
               casting='unsafe')
    list(st['tp'].map(get, range(NCORES)))


# --------------------------------------------------------------------------
# host pre/post processing
# --------------------------------------------------------------------------

def _prep_weights(inputs):
    Ws = np.asarray(inputs['Ws'], np.float32)
    bs = np.asarray(inputs['bs'], np.float32)
    Wq = np.asarray(inputs['Wq'], np.float32)
    bq = np.asarray(inputs['bq'], np.float32)
    Wk = np.asarray(inputs['Wk'], np.float32)
    Wout = np.asarray(inputs['Wout'], np.float32)
    bout = np.asarray(inputs['bout'], np.float32)
    scale = 1.0 / np.sqrt(np.float32(DG))
    w = {
        'wsv': np.ascontiguousarray(Ws[:, :OD].T).astype(BF16),
        'wq': np.concatenate([np.ascontiguousarray((Wq[d] * scale).T)
                              for d in range(DIRS)], 0).astype(BF16),
        'wk': np.concatenate([np.ascontiguousarray(Wk[d].T)
                              for d in range(DIRS)], 0).astype(BF16),
        'wo': np.concatenate([Wout[d].transpose(2, 0, 1).reshape(OD, OD)
                              for d in range(DIRS)], 0).astype(BF16),
        'bq': (bq.reshape(1, DIRS * OD) * scale).astype(BF16),
        'bo': bout.sum(0).reshape(1, OD).astype(BF16),
    }
    return w, Ws[:, OD:], bs


def kernel(**inputs) -> np.ndarray:
    _ensure_built()
    st = _state
    jax = st['jax']
    pool = st['pool']
    put = lambda arr: jax.device_put(arr, st['sharding'])

    v = np.asarray(inputs['v'], np.float32)
    pos = np.asarray(inputs['position_embedding'], np.float32)
    q = np.asarray(inputs['q'], np.float32)
    Wv = np.asarray(inputs['Wv'], np.float32)
    bv = np.asarray(inputs['bv'], np.float32)
    Wp = np.asarray(inputs['Wp'], np.float32)
    bp = np.asarray(inputs['bp'], np.float32)

    futs = {}

    # ---- weights: upload once, cache on device ----
    # change-detection key: f32 pairwise sums (memory-bound, ~20ms total)
    wkey = (float(np.asarray(inputs['Ws']).sum()),
            float(np.asarray(inputs['Wq']).sum()),
            float(np.asarray(inputs['Wk']).sum()),
            float(np.asarray(inputs['Wout']).sum()))
    if st.get('wkey') != wkey:
        w, Wsq, bs = _prep_weights(inputs)
        st['Wsq'] = Wsq
        st['bs'] = bs
        for name, arr in w.items():
            futs[name] = pool.submit(put, np.concatenate([arr] * NCORES, 0))
        st['wkey'] = wkey

    # resolve weight device arrays (first call only)
    wdev = st.setdefault('wdev', {})
    for name in ('wsv', 'wq', 'wk', 'wo', 'bq', 'bo'):
        if name in futs:
            wdev[name] = futs[name].result()

    # ---- q_s = q @ Ws_q.T + bs  (whole batch, tiny) ----
    qs_b = (q @ st['Wsq'].T + st['bs']).astype(BF16)

    # ---- chunked pipeline: host compute -> upload -> exec -> download ----
    Wp_all = Wp.reshape(DIRS * H, PD).T
    bp_row = bp.reshape(1, DIRS * H)
    nchunks = len(CHUNK_SIZES)
    result = np.empty((B, N, OD), np.float32)
    act_chunks = [None] * nchunks
    cfuts = [None] * nchunks

    RB = 2 * N + 1
    result2 = result.reshape(B * N, OD)

    def run_chunk(c, bc, fn, in_names, out_info, r0, fav, fx):
        args = {'av8': fav.result(), 'x': fx.result(), **wdev}
        scratch = st['scratch'][c]
        if scratch is None:
            scratch = tuple(
                jax.device_put(np.zeros((NCORES * shp[0],) + shp[1:], dtp),
                               st['sharding'])
                for shp, dtp in out_info)
        outs = fn(*[args[n] for n in in_names], *scratch)
        _fetch_add_sharded(outs[0], outs[1], act_chunks[c],
                           result2[r0:r0 + bc * N])
        st['scratch'][c] = outs

    # persistent host buffers (avoid per-call alloc + page faults); safe
    # to reuse: kernel() joins all pipeline work before returning.
    bufs = st.setdefault('hostbufs', {})

    def hbuf(key, shape, dtype):
        b = bufs.get(key)
        if b is None or b.shape != tuple(shape):
            b = np.empty(shape, dtype)
            bufs[key] = b
        return b

    b0 = 0
    for c, bc in enumerate(CHUNK_SIZES):
        bsl = slice(b0, b0 + bc)
        fn, in_names, out_info = st['fns'][bc // NCORES]
        X = hbuf(('X', c), (bc, RB, OD), BF16)
        # position projection chunk: [bc,N,NG,PD] -> [bc,N,(d,h),NG] bf16
        P = hbuf(('P', c), (bc * N * NG, DIRS * H), np.float32)
        np.matmul(pos[bsl].reshape(-1, PD), Wp_all, out=P)
        P += bp_row
        # direct strided cast-assign into the X view (single pass)
        X[:, 1:, :].reshape(bc, N, DIRS * H, NG)[...] = \
            P.reshape(bc, N, NG, DIRS * H).transpose(0, 1, 3, 2)
        X[:, 0, :] = qs_b[bsl]
        fx = pool.submit(_put_sharded, X.reshape(bc * RB, OD))
        # v transform chunk: act_v0 = relu(v @ Wv.T + bv), fp8 on the wire
        a = hbuf(('a', c), (bc * N, OD), np.float32)
        np.matmul(v[bsl].reshape(-1, VD), Wv.T, out=a)
        a += bv
        np.maximum(a, 0, out=a)                # [bc*N, OD] f32
        act_chunks[c] = a
        a8 = hbuf(('a8', c), (bc * N, OD), F8)
        a8[...] = a
        fav = pool.submit(_put_sharded, a8)
        cfuts[c] = st['runner'].submit(run_chunk, c, bc, fn, in_names,
                                       out_info, b0 * N, fav, fx)
        b0 += bc

    for c in range(nchunks):
        cfuts[c].result()
    return result


if __name__ == '__main__':
    rng = np.random.default_rng(0)
    ins = {
        'v': rng.standard_normal((B, N, VD)).astype(np.float32),
        'position_embedding': rng.random((B, N, NG, PD)).astype(np.float32),
        'q': rng.standard_normal((B, QD)).astype(np.float32),
        'Wv': 0.02 * rng.standard_normal((OD, VD)).astype(np.float32),
        'bv': np.zeros(OD, np.float32),
        'Ws': 0.02 * rng.standard_normal((OD, OD + QD)).astype(np.float32),
        'bs': np.zeros(OD, np.float32),
        'Wb': 0.02 * rng.standard_normal((1, 1)).astype(np.float32),
        'bb': np.zeros(1, np.float32),
        'Wq': 0.02 * rng.standard_normal((DIRS, OD, OD)).astype(np.float32),
        'bq': np.zeros((DIRS, OD), np.float32),
        'Wk': 0.02 * rng.standard_normal((DIRS, OD, OD)).astype(np.float32),
        'bk': np.zeros((DIRS, OD), np.float32),
        'Wp': 0.02 * rng.standard_normal((DIRS, H, PD)).astype(np.float32),
        'bp': np.zeros((DIRS, H), np.float32),
        'Wout': 0.02 * rng.standard_normal((DIRS, H, DG, OD)).astype(np.float32),
        'bout': np.zeros((DIRS, OD), np.float32),
    }
    out = kernel(**ins)
    print('kernel output', out.shape, out.dtype, float(np.abs(out).mean()))
